# revision 22
# baseline (speedup 1.0000x reference)
"""Trainium2 Bass kernel for a dense transformer block (pre-LN causal MHA + FFN).

Sharding: B=2 batches -> 2 groups of 4 cores ([0-3]=batch0, [4-7]=batch1).
Within a group, core j owns query chunks (j, 7-j) of 256 rows (load-balanced
causal attention).  Each core computes K/V for its own 512 tokens (all heads);
K/V are exchanged with one AllGather per group.  Everything else (LN1, QKV,
attention, output projections, LN2, FFN) is computed per-core on its own rows
with no further communication.  All activations live in transposed ("T")
layout [feature, token] so matmul contractions never need on-device
transposes; the host ships x pre-transposed and transposes the output back.
Matmuls run in bf16 with f32 PSUM accumulation; softmax uses exp without
max-subtraction (scores are bounded by LN) and multiplicative 0/1 masks.
"""

import sys

sys.path.insert(0, "/opt/trn_rl_repo")

import numpy as np
import ml_dtypes

import concourse.bass as bass
import concourse.tile as tile
from concourse import bacc, mybir
from concourse.bass_utils import run_bass_kernel_spmd

BF16 = ml_dtypes.bfloat16
P = 128
import os
N_AG = int(os.environ.get("N_AG", "1"))


class Cfg:
    def __init__(self, B=2, S=2048, D=2048, H=16, DF=8192):
        self.B, self.S, self.D, self.H, self.DF = B, S, D, H, DF
        self.HD = D // H
        assert self.HD == P
        self.DT = D // P          # feature tiles of D
        self.FT = DF // P         # feature tiles of DF
        self.SP = S // P          # kv tiles (of 128 tokens)
        self.NCH = 8              # chunks per batch (2 per core x 4 cores)
        self.CH = S // self.NCH   # chunk length (256 full-size)
        self.QF = 2 * self.CH     # tokens per core
        self.TPC = self.QF // P   # token tiles per core
        self.CHP = self.CH // P   # kv tiles per chunk
        self.NE = D // 512        # 512-wide slices of D
        # kv allgather buffer layout (bf16 elements)
        self.KOFF = 0
        self.KLEN = H * self.HD * self.QF
        self.VOFF = self.KLEN
        self.VLEN = self.QF * D
        self.KVN = self.KLEN + self.VLEN


def chunk_rank(c):
    return c if c < 4 else 7 - c


def chunk_slot(c):
    return 0 if c < 4 else 1


def build_program(cfg: Cfg):
    """Build the SPMD Bacc program (same on all 8 cores)."""
    c = cfg
    nc = bacc.Bacc("TRN2", num_devices=8)
    f32 = mybir.dt.float32
    bf = mybir.dt.bfloat16

    xT_d = nc.dram_tensor("xT", [c.D, c.QF], f32, kind="ExternalInput")
    mask_d = nc.dram_tensor("mask", [2, c.SP, P, c.CH], bf, kind="ExternalInput")
    wq_d = nc.dram_tensor("wq", [c.H, c.DT, P, c.HD], bf, kind="ExternalInput")
    wk_d = nc.dram_tensor("wk", [c.H, c.DT, P, c.HD], bf, kind="ExternalInput")
    wv_d = nc.dram_tensor("wv", [c.DT, P, c.D], bf, kind="ExternalInput")
    wo1_d = nc.dram_tensor("wo1", [c.DT, c.H, c.HD, P], bf, kind="ExternalInput")
    wo2_d = nc.dram_tensor("wo2", [c.DT, c.DT, P, P], bf, kind="ExternalInput")
    fw1_d = nc.dram_tensor("fw1", [c.FT, c.DT, P, P], bf, kind="ExternalInput")
    fw2_d = nc.dram_tensor("fw2", [c.DT, c.FT, P, P], bf, kind="ExternalInput")
    ln1s_d = nc.dram_tensor("ln1s", [P, c.DT], f32, kind="ExternalInput")
    ln1b_d = nc.dram_tensor("ln1b", [P, c.DT], f32, kind="ExternalInput")
    ln2s_d = nc.dram_tensor("ln2s", [P, c.DT], f32, kind="ExternalInput")
    ln2b_d = nc.dram_tensor("ln2b", [P, c.DT], f32, kind="ExternalInput")
    fb1_d = nc.dram_tensor("fb1", [P, c.FT], f32, kind="ExternalInput")
    fb2_d = nc.dram_tensor("fb2", [P, c.DT], f32, kind="ExternalInput")
    outT_d = nc.dram_tensor("outT", [c.D, c.QF], f32, kind="ExternalOutput")

    AF = mybir.ActivationFunctionType
    OP = mybir.AluOpType

    def bcast_ap(dram_ap, parts=P):
        # [1, N] DRAM tile -> stride-0 partition broadcast [parts, N]
        return bass.AP(
            tensor=dram_ap.tensor,
            offset=dram_ap.offset,
            ap=[[0, parts], list(dram_ap.ap[-1])],
        )

    from contextlib import ExitStack
    with tile.TileContext(nc) as tc:
        with ExitStack() as _ctx:
            def _pool(name, bufs, space="SBUF"):
                return _ctx.enter_context(
                    tc.tile_pool(name=name, bufs=bufs, space=space))
            singles = _pool("singles", 1)
            acts = _pool("acts", 1)
            streams = _pool("streams", 4)
            fw2s = _pool("fw2s", 3)
            wvs = _pool("wvs", 1)
            kvs = _pool("kvs", 2)
            etile = _pool("etile", 6)
            drains = _pool("drains", 3)
            xs = _pool("xs", 3)
            bcasts = _pool("bcasts", 1)
            smalls = _pool("smalls", 1)
            recs = _pool("recs", 2)
            ps512 = _pool("ps512", 3, "PSUM")
            pssc = _pool("pssc", 2, "PSUM")
            psat = _pool("psat", 2, "PSUM")
            psvec = _pool("psvec", 1, "PSUM")
            dram = _pool("dram", 1, "DRAM")
            drsc = _pool("drsc", 4, "DRAM")

            # ---- constants ----
            ones_f = singles.tile([P, 1], f32)
            nc.vector.memset(ones_f, 1.0)
            ones_b = singles.tile([P, 1], bf)
            nc.vector.memset(ones_b, 1.0)
            eps_t = singles.tile([1, 1], f32)
            nc.vector.memset(eps_t, 1e-6)
            ones_r = singles.tile([1, P], f32)
            nc.vector.memset(ones_r, 1.0)
            ln1s = singles.tile([P, c.DT], f32); nc.sync.dma_start(ln1s[:], ln1s_d[:])
            ln1b = singles.tile([P, c.DT], f32); nc.sync.dma_start(ln1b[:], ln1b_d[:])
            ln2s = singles.tile([P, c.DT], f32); nc.sync.dma_start(ln2s[:], ln2s_d[:])
            ln2b = singles.tile([P, c.DT], f32); nc.sync.dma_start(ln2b[:], ln2b_d[:])
            fb1 = singles.tile([P, c.FT], f32); nc.sync.dma_start(fb1[:], fb1_d[:])
            fb2 = singles.tile([P, c.DT], f32); nc.sync.dma_start(fb2[:], fb2_d[:])
            mask_sb = singles.tile([P, 2, c.SP, c.CH], bf)
            nc.sync.dma_start(mask_sb[:], mask_d[:].rearrange("c k p q -> p c k q"))

            def emit_ln(src_rows, dst_bf, s_sb, b_sb):
                """dst = LN(src) over features; src_rows(dt) -> DRAM AP [P, QF]."""
                mu_t = ps512.tile([P, c.QF], f32, tag="mm")
                mu_ps = mu_t[0:1, :]
                m2_t = ps512.tile([P, c.QF], f32, tag="mm")
                m2_ps = m2_t[0:1, :]
                xtiles = []
                for dt in range(c.DT):
                    xt = xs.tile([P, c.QF], f32, tag="xs")
                    nc.sync.dma_start(xt[:], src_rows(dt))
                    sqt = xs.tile([P, c.QF], f32, tag="sqt")
                    nc.vector.tensor_mul(sqt[:], xt[:], xt[:])
                    nc.tensor.matmul(mu_ps, ones_f, xt[:],
                                     start=(dt == 0), stop=(dt == c.DT - 1))
                    nc.tensor.matmul(m2_ps, ones_f, sqt[:],
                                     start=(dt == 0), stop=(dt == c.DT - 1))
                mu = smalls.tile([1, c.QF], f32, tag="mu")
                m2 = smalls.tile([1, c.QF], f32, tag="m2")
                nc.vector.tensor_scalar_mul(mu[:], mu_ps[:], 1.0 / c.D)
                nc.vector.tensor_scalar_mul(m2[:], m2_ps[:], 1.0 / c.D)
                var = smalls.tile([1, c.QF], f32, tag="var")
                nc.vector.tensor_mul(var[:], mu[:], mu[:])
                nc.vector.tensor_tensor(var[:], m2[:], var[:], OP.subtract)
                std = smalls.tile([1, c.QF], f32, tag="std")
                nc.scalar.activation(std[:], var[:], AF.Sqrt, bias=eps_t[:])
                rstd = smalls.tile([1, c.QF], f32, tag="rstd")
                nc.vector.reciprocal(rstd[:], std[:])
                mu_bp = ps512.tile([P, c.QF], f32, tag="mm")
                nc.tensor.matmul(mu_bp, ones_r, mu[:], start=True, stop=True)
                mu_b = bcasts.tile([P, c.QF], f32, tag="mu_b")
                nc.any.tensor_copy(out=mu_b[:], in_=mu_bp[:])
                rs_bp = ps512.tile([P, c.QF], f32, tag="mm")
                nc.tensor.matmul(rs_bp, ones_r, rstd[:], start=True, stop=True)
                rs_b = bcasts.tile([P, c.QF], f32, tag="rs_b")
                nc.any.tensor_copy(out=rs_b[:], in_=rs_bp[:])
                for dt in range(c.DT):
                    xt = xs.tile([P, c.QF], f32, tag="xs")
                    nc.sync.dma_start(xt[:], src_rows(dt))
                    nc.vector.tensor_tensor(xt[:], xt[:], mu_b[:], OP.subtract)
                    nc.vector.tensor_tensor(xt[:], xt[:], rs_b[:], OP.mult)
                    nc.vector.tensor_scalar(
                        out=dst_bf[:, dt, :], in0=xt[:],
                        scalar1=s_sb[:, dt:dt + 1], scalar2=b_sb[:, dt:dt + 1],
                        op0=OP.mult, op1=OP.add)

            # ---- phase 0: LN1 -> hT ----
            xT_t = xT_d[:].rearrange("(dt p) t -> dt p t", p=P)
            hT = acts.tile([P, c.DT, c.QF], bf, tag="hT")
            emit_ln(lambda dt: xT_t[dt], hT, ln1s, ln1b)

            # ---- kv exchange buffers: 4 head-groups, pipelined AGs ----
            HPG = c.H // 4                  # heads per AG group
            VW = HPG * c.HD                 # V column width per group
            GK = HPG * c.HD * c.QF          # K elems per group
            GV = c.QF * VW                  # V elems per group
            GLEN = GK + GV
            SGPA = 4 // N_AG                 # subgroups per AG
            kv_ins = [dram.tile([1, SGPA * GLEN], bf, tag=f"kvi{a}",
                                name=f"kvi{a}") for a in range(N_AG)]
            kv_outs = [dram.tile([4, SGPA * GLEN], bf, tag=f"kvo{a}",
                                 name=f"kvo{a}") for a in range(N_AG)]
            kvi = lambda g: kv_ins[g // SGPA][0][(g % SGPA) * GLEN:
                                                 (g % SGPA + 1) * GLEN]
            kvo = lambda g: kv_outs[g // SGPA][:, (g % SGPA) * GLEN:
                                              (g % SGPA + 1) * GLEN]

            x1_dr = dram.tile([c.DT, P, c.QF], f32, tag="x1dr")
            f_dr = dram.tile([c.DT, P, c.QF], f32, tag="fdr")

            # ---- phase 1+2+3: per head-group K, V, then AllGather ----
            for g in range(4):
                for hh in range(HPG):
                    h = HPG * g + hh
                    wk_sb = streams.tile([P, c.DT, c.HD], bf, tag="wstream")
                    nc.sync.dma_start(wk_sb[:],
                                      wk_d[h].rearrange("dt p hd -> p dt hd"))
                    kps = ps512.tile([P, c.QF], f32, tag="mm")
                    for dt in range(c.DT):
                        nc.tensor.matmul(kps, wk_sb[:, dt, :], hT[:, dt, :],
                                         start=(dt == 0), stop=(dt == c.DT - 1))
                    k_tmp = drains.tile([P, c.QF], bf, tag="ktmp")
                    nc.any.tensor_copy(out=k_tmp[:], in_=kps[:])
                    ksec = kvi(g)[hh * c.HD * c.QF:(hh + 1) * c.HD * c.QF]
                    nc.sync.dma_start(ksec.rearrange("(p t) -> p t", p=P), k_tmp[:])
                # V columns for this head group (512 wide)
                wv_sb = wvs.tile([P, c.DT, VW], bf, tag="wvhc")
                nc.scalar.dma_start(
                    wv_sb[:],
                    wv_d[:, :, g * VW:(g + 1) * VW].rearrange("dt p e -> p dt e"))
                vsec = kvi(g)[GK:GK + GV]
                vsec_t = vsec.rearrange("(tt p e) -> p tt e", p=P, tt=c.TPC)
                for tt in range(c.TPC):
                    vps = ps512.tile([P, VW], f32, tag="mm")
                    for dt in range(c.DT):
                        nc.tensor.matmul(vps, hT[:, dt, tt * P:(tt + 1) * P],
                                         wv_sb[:, dt, :],
                                         start=(dt == 0), stop=(dt == c.DT - 1))
                    v_dr = drains.tile([P, VW], bf, tag="vdr")
                    nc.any.tensor_copy(out=v_dr[:], in_=vps[:])
                    nc.sync.dma_start(vsec_t[:, tt, :], v_dr[:])
                if (g + 1) % SGPA == 0:
                    a = g // SGPA
                    nc.gpsimd.collective_compute(
                        "AllGather", OP.bypass,
                        replica_groups=[[0, 1, 2, 3], [4, 5, 6, 7]],
                        ins=[kv_ins[a].opt()],
                        outs=[kv_outs[a].opt()],
                    )

            # ---- phase 3b: Q projection (overlaps the AllGathers) ----
            qT_all = acts.tile([P, c.H, c.QF], bf, tag="qo")
            for h in range(c.H):
                wq_sb = streams.tile([P, c.DT, c.HD], bf, tag="wstream")
                nc.sync.dma_start(wq_sb[:], wq_d[h].rearrange("dt p hd -> p dt hd"))
                qps = ps512.tile([P, c.QF], f32, tag="mm")
                for dt in range(c.DT):
                    nc.tensor.matmul(qps, wq_sb[:, dt, :], hT[:, dt, :],
                                     start=(dt == 0), stop=(dt == c.DT - 1))
                nc.vector.tensor_copy(out=qT_all[:, h, :], in_=qps[:])

            # ---- phase 4: attention ----
            attn_all = acts.tile([P, c.H, c.QF], bf, tag="ah")
            isq = float(c.HD) ** -0.5
            for h in range(c.H):
                g, hh = h // HPG, h % HPG
                k_sb = kvs.tile([P, 4, c.QF], bf, tag="ksb")
                ksec_o = kvo(g)[:, hh * c.HD * c.QF:(hh + 1) * c.HD * c.QF]
                nc.sync.dma_start(k_sb[:],
                                  ksec_o.rearrange("r (p t) -> p r t", p=P))
                v_sb = kvs.tile([P, 4, c.TPC, c.HD], bf, tag="vsb")
                for r in range(4):
                    vsec_o = kvo(g)[r, GK:GK + GV]
                    nc.scalar.dma_start(
                        v_sb[:, r],
                        vsec_o.rearrange("(tt p e) -> p tt e", p=P, tt=c.TPC)
                        [:, :, hh * c.HD:(hh + 1) * c.HD])
                for ci in range(2):
                    qs = qT_all[:, h, ci * c.CH:(ci + 1) * c.CH]
                    aps = psat.tile([P, c.CH], f32, tag="at")
                    sps = psvec.tile([1, c.CH], f32, tag="vec")
                    nkt = c.SP // 2 if ci == 0 else c.SP
                    for kt in range(nkt):
                        ch = kt // c.CHP
                        r = chunk_rank(ch)
                        sl = chunk_slot(ch)
                        kin = kt % c.CHP
                        toff = sl * c.CH + kin * P
                        scps = pssc.tile([P, c.CH], f32, tag="sc")
                        nc.tensor.matmul(scps, k_sb[:, r, toff:toff + P], qs,
                                         start=True, stop=True)
                        e_bf = etile.tile([P, c.CH], bf, tag="ebf")
                        nc.scalar.activation(e_bf[:], scps[:], AF.Exp, scale=isq)
                        nc.vector.tensor_mul(e_bf[:], e_bf[:], mask_sb[:, ci, kt, :])
                        vidx = sl * c.CHP + kin
                        nc.tensor.matmul(aps, v_sb[:, r, vidx, :], e_bf[:],
                                         start=(kt == 0), stop=(kt == nkt - 1))
                        nc.tensor.matmul(sps, ones_b, e_bf[:],
                                         start=(kt == 0), stop=(kt == nkt - 1))
                    rec = recs.tile([1, c.CH], f32, tag="rec")
                    nc.vector.reciprocal(rec[:], sps[:])
                    rec_bp = pssc.tile([P, c.CH], f32, tag="sc")
                    nc.tensor.matmul(rec_bp, ones_r, rec[:], start=True, stop=True)
                    rec_b = bcasts.tile([P, c.CH], f32, tag="rec_b")
                    nc.any.tensor_copy(out=rec_b[:], in_=rec_bp[:])
                    nc.vector.tensor_tensor(
                        attn_all[:, h, ci * c.CH:(ci + 1) * c.CH],
                        aps[:], rec_b[:], OP.mult)

            # ---- phase 5: o1T = sum_h wo1_h^T @ attnT_h ----
            o1T = acts.tile([P, c.DT, c.QF], bf, tag="qo")
            for dt in range(c.DT):
                w_sb = streams.tile([P, c.H, P], bf, tag="wstream")
                nc.scalar.dma_start(w_sb[:], wo1_d[dt].rearrange("h hd e -> hd h e"))
                ops = ps512.tile([P, c.QF], f32, tag="mm")
                for h in range(c.H):
                    nc.tensor.matmul(ops, w_sb[:, h, :], attn_all[:, h, :],
                                     start=(h == 0), stop=(h == c.H - 1))
                nc.any.tensor_copy(out=o1T[:, dt, :], in_=ops[:])

            # ---- phase 6: o2T + residual -> x1 (DRAM) ----
            for et in range(c.DT):
                w_sb = streams.tile([P, c.DT, P], bf, tag="wstream")
                nc.scalar.dma_start(w_sb[:], wo2_d[et].rearrange("dt d e -> d dt e"))
                ops = ps512.tile([P, c.QF], f32, tag="mm")
                for dt in range(c.DT):
                    nc.tensor.matmul(ops, w_sb[:, dt, :], o1T[:, dt, :],
                                     start=(dt == 0), stop=(dt == c.DT - 1))
                x1_t = drains.tile([P, c.QF], f32, tag="x1t")
                nc.vector.tensor_tensor(x1_t[:], ops[:], hT[:, et, :], OP.add)
                nc.sync.dma_start(x1_dr[et], x1_t[:])

            # ---- phase 7: LN2 -> h2T ----
            h2T = acts.tile([P, c.DT, c.QF], bf, tag="ah")
            emit_ln(lambda dt: x1_dr[dt], h2T, ln2s, ln2b)

            # ---- phases 8+9: FFN by d_ff halves (N=512, partial-f via DRAM) ----
            outT_t = outT_d[:].rearrange("(et p) t -> et p t", p=P)
            FH = c.FT // 2
            FQ = min(16, FH)
            for fh in range(2):
                zT = acts.tile([P, FH, c.QF], bf, tag="big")
                for fi in range(FH):
                    ft = fh * FH + fi
                    w_sb = streams.tile([P, c.DT, P], bf, tag="wstream")
                    nc.scalar.dma_start(w_sb[:],
                                        fw1_d[ft].rearrange("dt d f -> d dt f"))
                    zps = ps512.tile([P, c.QF], f32, tag="mm")
                    for dt in range(c.DT):
                        nc.tensor.matmul(zps, w_sb[:, dt, :], h2T[:, dt, :],
                                         start=(dt == 0), stop=(dt == c.DT - 1))
                    nc.scalar.activation(zT[:, fi, :], zps[:], AF.Relu,
                                         bias=fb1[:, ft:ft + 1])
                for et in range(c.DT):
                    fps = ps512.tile([P, c.QF], f32, tag="mm")
                    for qt in range(FH // FQ):
                        w_sb = fw2s.tile([P, FQ, P], bf, tag="fw2s")
                        nc.sync.dma_start(
                            w_sb[:],
                            fw2_d[et, fh * FH + qt * FQ:fh * FH + (qt + 1) * FQ]
                            .rearrange("ft f e -> f ft e"))
                        for qi in range(FQ):
                            fi = qt * FQ + qi
                            nc.tensor.matmul(fps, w_sb[:, qi, :], zT[:, fi, :],
                                             start=(fi == 0),
                                             stop=(fi == FH - 1))
                    if fh == 0:
                        fp_t = drains.tile([P, c.QF], f32, tag="odrain")
                        nc.any.tensor_copy(out=fp_t[:], in_=fps[:])
                        nc.sync.dma_start(f_dr[et], fp_t[:])
                    else:
                        fpart = xs.tile([P, c.QF], f32, tag="xs")
                        nc.sync.dma_start(fpart[:], f_dr[et])
                        tmp_o = drains.tile([P, c.QF], f32, tag="odrain")
                        nc.vector.tensor_scalar(out=tmp_o[:], in0=fps[:],
                                                scalar1=fb2[:, et:et + 1],
                                                scalar2=None, op0=OP.add)
                        nc.vector.tensor_tensor(tmp_o[:], tmp_o[:], fpart[:],
                                                OP.add)
                        nc.vector.tensor_tensor(tmp_o[:], tmp_o[:],
                                                h2T[:, et, :], OP.add)
                        nc.sync.dma_start(outT_t[et], tmp_o[:])

    nc.compile()
    return nc


def prepare_inputs(cfg: Cfg, inputs):
    """Full inputs -> per-core in_maps."""
    c = cfg
    x = np.asarray(inputs["x"], np.float32)
    wq = np.asarray(inputs["wq"], np.float32).reshape(c.D, c.D)
    wk = np.asarray(inputs["wk"], np.float32).reshape(c.D, c.D)
    wv = np.asarray(inputs["wv"], np.float32).reshape(c.D, c.D)
    wo1 = np.asarray(inputs["wo1"], np.float32)   # [H, HD, D]
    wo2 = np.asarray(inputs["wo2"], np.float32)
    fw1 = np.asarray(inputs["fw1"], np.float32)
    fw2 = np.asarray(inputs["fw2"], np.float32)

    def pt(a):  # [D, X] -> [DT, P, X] tiles, bf16
        return np.ascontiguousarray(a.reshape(-1, P, a.shape[1])).astype(BF16)

    shared = {
        "wq": np.ascontiguousarray(
            wq.reshape(c.DT, P, c.H, c.HD).transpose(2, 0, 1, 3)).astype(BF16),
        "wk": np.ascontiguousarray(
            wk.reshape(c.DT, P, c.H, c.HD).transpose(2, 0, 1, 3)).astype(BF16),
        "wv": pt(wv),
        "wo1": np.ascontiguousarray(
            wo1.reshape(c.H, c.HD, c.DT, P).transpose(2, 0, 1, 3)).astype(BF16),
        "wo2": np.ascontiguousarray(
            wo2.reshape(c.DT, P, c.DT, P).transpose(2, 0, 1, 3)).astype(BF16),
        "fw1": np.ascontiguousarray(
            fw1.reshape(c.DT, P, c.FT, P).transpose(2, 0, 1, 3)).astype(BF16),
        "fw2": np.ascontiguousarray(
            fw2.reshape(c.FT, P, c.DT, P).transpose(2, 0, 1, 3)).astype(BF16),
        "ln1s": np.ascontiguousarray(
            np.asarray(inputs["ln1_scale"], np.float32).reshape(c.DT, P).T),
        "ln1b": np.ascontiguousarray(
            np.asarray(inputs["ln1_bias"], np.float32).reshape(c.DT, P).T),
        "ln2s": np.ascontiguousarray(
            np.asarray(inputs["ln2_scale"], np.float32).reshape(c.DT, P).T),
        "ln2b": np.ascontiguousarray(
            np.asarray(inputs["ln2_bias"], np.float32).reshape(c.DT, P).T),
        "fb1": np.ascontiguousarray(
            np.asarray(inputs["fb1"], np.float32).reshape(c.FT, P).T),
        "fb2": np.ascontiguousarray(
            np.asarray(inputs["fb2"], np.float32).reshape(c.DT, P).T),
    }

    in_maps = []
    for core in range(8):
        b, j = core // 4, core % 4
        ca, cb = j, 7 - j
        rows_a = slice(c.CH * ca, c.CH * (ca + 1))
        rows_b = slice(c.CH * cb, c.CH * (cb + 1))
        xc = np.concatenate([x[b, rows_a], x[b, rows_b]], axis=0)  # [QF, D]
        xT = np.ascontiguousarray(xc.T)
        mask = np.zeros((2, c.SP, P, c.CH), np.float32)
        for ci, ch in enumerate((ca, cb)):
            q0 = c.CH * ch
            qpos = q0 + np.arange(c.CH)[None, :]
            for kt in range(c.SP):
                kvpos = kt * P + np.arange(P)[:, None]
                mask[ci, kt] = (kvpos <= qpos)
        m = dict(shared)
        m["xT"] = xT
        m["mask"] = mask.astype(BF16)
        in_maps.append(m)
    return in_maps


def assemble_output(cfg: Cfg, results):
    c = cfg
    out = np.empty((c.B, c.S, c.D), np.float32)
    for core in range(8):
        b, j = core // 4, core % 4
        oT = results[core]["outT"]  # [D, QF]
        out[b, c.CH * j:c.CH * (j + 1)] = oT[:, :c.CH].T
        out[b, c.CH * (7 - j):c.CH * (8 - j)] = oT[:, c.CH:].T
    return out


_CACHE = {}


def kernel(**inputs):
    cfg = Cfg()
    if "nc" not in _CACHE:
        _CACHE["nc"] = build_program(cfg)
    nc = _CACHE["nc"]
    in_maps = prepare_inputs(cfg, inputs)
    res = run_bass_kernel_spmd(nc, in_maps, core_ids=list(range(8)))
    return assemble_output(cfg, res.results)


# revision 23
# speedup vs baseline: 1.3923x; 1.3923x over previous
"""Trainium2 Bass kernel for a dense transformer block (pre-LN causal MHA + FFN).

Sharding: B=2 batches -> 2 groups of 4 cores ([0-3]=batch0, [4-7]=batch1).
Within a group, core j owns query chunks (j, 7-j) of 256 rows (load-balanced
causal attention).  Each core computes K/V for its own 512 tokens (all heads);
K/V are exchanged with one AllGather per group.  Everything else (LN1, QKV,
attention, output projections, LN2, FFN) is computed per-core on its own rows
with no further communication.  All activations live in transposed ("T")
layout [feature, token] so matmul contractions never need on-device
transposes; the host ships x pre-transposed and transposes the output back.
Matmuls run in bf16 with f32 PSUM accumulation; softmax uses exp without
max-subtraction (scores are bounded by LN) and multiplicative 0/1 masks.
"""

import sys

sys.path.insert(0, "/opt/trn_rl_repo")

import numpy as np
import ml_dtypes

import concourse.bass as bass
import concourse.tile as tile
from concourse import bacc, mybir
from concourse.bass_utils import run_bass_kernel_spmd

BF16 = ml_dtypes.bfloat16
P = 128
import os
N_AG = int(os.environ.get("N_AG", "1"))


class Cfg:
    def __init__(self, B=2, S=2048, D=2048, H=16, DF=8192):
        self.B, self.S, self.D, self.H, self.DF = B, S, D, H, DF
        self.HD = D // H
        assert self.HD == P
        self.DT = D // P          # feature tiles of D
        self.FT = DF // P         # feature tiles of DF
        self.SP = S // P          # kv tiles (of 128 tokens)
        self.NCH = 8              # chunks per batch (2 per core x 4 cores)
        self.CH = S // self.NCH   # chunk length (256 full-size)
        self.QF = 2 * self.CH     # tokens per core
        self.TPC = self.QF // P   # token tiles per core
        self.CHP = self.CH // P   # kv tiles per chunk
        self.NE = D // 512        # 512-wide slices of D
        # kv allgather buffer layout (bf16 elements)
        self.KOFF = 0
        self.KLEN = H * self.HD * self.QF
        self.VOFF = self.KLEN
        self.VLEN = self.QF * D
        self.KVN = self.KLEN + self.VLEN


def chunk_rank(c):
    return c if c < 4 else 7 - c


def chunk_slot(c):
    return 0 if c < 4 else 1


def build_program(cfg: Cfg):
    """Build the SPMD Bacc program (same on all 8 cores)."""
    c = cfg
    nc = bacc.Bacc("TRN2", num_devices=8)
    f32 = mybir.dt.float32
    bf = mybir.dt.bfloat16

    xT_d = nc.dram_tensor("xT", [c.D, c.QF], f32, kind="ExternalInput")
    mask_d = nc.dram_tensor("mask", [2, c.SP, P, c.CH], bf, kind="ExternalInput")
    wq_d = nc.dram_tensor("wq", [c.H, c.DT, P, c.HD], bf, kind="ExternalInput")
    wk_d = nc.dram_tensor("wk", [c.H, c.DT, P, c.HD], bf, kind="ExternalInput")
    wv_d = nc.dram_tensor("wv", [c.DT, P, c.D], bf, kind="ExternalInput")
    wo1_d = nc.dram_tensor("wo1", [c.DT, c.H, c.HD, P], bf, kind="ExternalInput")
    wo2_d = nc.dram_tensor("wo2", [c.DT, c.DT, P, P], bf, kind="ExternalInput")
    fw1_d = nc.dram_tensor("fw1", [c.FT, c.DT, P, P], bf, kind="ExternalInput")
    fw2_d = nc.dram_tensor("fw2", [c.DT, c.FT, P, P], bf, kind="ExternalInput")
    ln1s_d = nc.dram_tensor("ln1s", [P, c.DT], f32, kind="ExternalInput")
    ln1b_d = nc.dram_tensor("ln1b", [P, c.DT], f32, kind="ExternalInput")
    ln2s_d = nc.dram_tensor("ln2s", [P, c.DT], f32, kind="ExternalInput")
    ln2b_d = nc.dram_tensor("ln2b", [P, c.DT], f32, kind="ExternalInput")
    fb1_d = nc.dram_tensor("fb1", [P, c.FT], f32, kind="ExternalInput")
    fb2_d = nc.dram_tensor("fb2", [P, c.DT], f32, kind="ExternalInput")
    outT_d = nc.dram_tensor("outT", [c.D, c.QF], f32, kind="ExternalOutput")

    AF = mybir.ActivationFunctionType
    OP = mybir.AluOpType

    def bcast_ap(dram_ap, parts=P):
        # [1, N] DRAM tile -> stride-0 partition broadcast [parts, N]
        return bass.AP(
            tensor=dram_ap.tensor,
            offset=dram_ap.offset,
            ap=[[0, parts], list(dram_ap.ap[-1])],
        )

    from contextlib import ExitStack
    with tile.TileContext(nc) as tc:
        with ExitStack() as _ctx:
            def _pool(name, bufs, space="SBUF"):
                return _ctx.enter_context(
                    tc.tile_pool(name=name, bufs=bufs, space=space))
            singles = _pool("singles", 1)
            acts = _pool("acts", 1)
            streams = _pool("streams", 4)
            fw2s = _pool("fw2s", 3)
            wvs = _pool("wvs", 1)
            kvs = _pool("kvs", 2)
            etile = _pool("etile", 6)
            drains = _pool("drains", 3)
            xs = _pool("xs", 3)
            bcasts = _pool("bcasts", 1)
            smalls = _pool("smalls", 1)
            recs = _pool("recs", 2)
            ps512 = _pool("ps512", 3, "PSUM")
            pssc = _pool("pssc", 2, "PSUM")
            psat = _pool("psat", 2, "PSUM")
            psvec = _pool("psvec", 1, "PSUM")
            dram = _pool("dram", 1, "DRAM")
            drsc = _pool("drsc", 4, "DRAM")

            # ---- constants ----
            ones_f = singles.tile([P, 1], f32)
            nc.vector.memset(ones_f, 1.0)
            ones_b = singles.tile([P, 1], bf)
            nc.vector.memset(ones_b, 1.0)
            eps_t = singles.tile([1, 1], f32)
            nc.vector.memset(eps_t, 1e-6)
            ones_r = singles.tile([1, P], f32)
            nc.vector.memset(ones_r, 1.0)
            ln1s = singles.tile([P, c.DT], f32); nc.sync.dma_start(ln1s[:], ln1s_d[:])
            ln1b = singles.tile([P, c.DT], f32); nc.sync.dma_start(ln1b[:], ln1b_d[:])
            ln2s = singles.tile([P, c.DT], f32); nc.sync.dma_start(ln2s[:], ln2s_d[:])
            ln2b = singles.tile([P, c.DT], f32); nc.sync.dma_start(ln2b[:], ln2b_d[:])
            fb1 = singles.tile([P, c.FT], f32); nc.sync.dma_start(fb1[:], fb1_d[:])
            fb2 = singles.tile([P, c.DT], f32); nc.sync.dma_start(fb2[:], fb2_d[:])
            mask_sb = singles.tile([P, 2, c.SP, c.CH], bf)
            nc.sync.dma_start(mask_sb[:], mask_d[:].rearrange("c k p q -> p c k q"))

            def emit_ln(src_rows, dst_bf, s_sb, b_sb):
                """dst = LN(src) over features; src_rows(dt) -> DRAM AP [P, QF]."""
                mu_t = ps512.tile([P, c.QF], f32, tag="mm")
                mu_ps = mu_t[0:1, :]
                m2_t = ps512.tile([P, c.QF], f32, tag="mm")
                m2_ps = m2_t[0:1, :]
                xtiles = []
                for dt in range(c.DT):
                    xt = xs.tile([P, c.QF], f32, tag="xs")
                    (nc.sync if dt % 2 == 0 else nc.scalar).dma_start(
                        xt[:], src_rows(dt))
                    sqt = xs.tile([P, c.QF], f32, tag="sqt")
                    nc.vector.tensor_mul(sqt[:], xt[:], xt[:])
                    nc.tensor.matmul(mu_ps, ones_f, xt[:],
                                     start=(dt == 0), stop=(dt == c.DT - 1))
                    nc.tensor.matmul(m2_ps, ones_f, sqt[:],
                                     start=(dt == 0), stop=(dt == c.DT - 1))
                mu = smalls.tile([1, c.QF], f32, tag="mu")
                m2 = smalls.tile([1, c.QF], f32, tag="m2")
                nc.vector.tensor_scalar_mul(mu[:], mu_ps[:], 1.0 / c.D)
                nc.vector.tensor_scalar_mul(m2[:], m2_ps[:], 1.0 / c.D)
                var = smalls.tile([1, c.QF], f32, tag="var")
                nc.vector.tensor_mul(var[:], mu[:], mu[:])
                nc.vector.tensor_tensor(var[:], m2[:], var[:], OP.subtract)
                std = smalls.tile([1, c.QF], f32, tag="std")
                nc.scalar.activation(std[:], var[:], AF.Sqrt, bias=eps_t[:])
                rstd = smalls.tile([1, c.QF], f32, tag="rstd")
                nc.vector.reciprocal(rstd[:], std[:])
                mu_bp = ps512.tile([P, c.QF], f32, tag="mm")
                nc.tensor.matmul(mu_bp, ones_r, mu[:], start=True, stop=True)
                mu_b = bcasts.tile([P, c.QF], f32, tag="mu_b")
                nc.any.tensor_copy(out=mu_b[:], in_=mu_bp[:])
                rs_bp = ps512.tile([P, c.QF], f32, tag="mm")
                nc.tensor.matmul(rs_bp, ones_r, rstd[:], start=True, stop=True)
                rs_b = bcasts.tile([P, c.QF], f32, tag="rs_b")
                nc.any.tensor_copy(out=rs_b[:], in_=rs_bp[:])
                for dt in range(c.DT):
                    xt = xs.tile([P, c.QF], f32, tag="xs")
                    (nc.sync if dt % 2 == 0 else nc.scalar).dma_start(
                        xt[:], src_rows(dt))
                    nc.vector.tensor_tensor(xt[:], xt[:], mu_b[:], OP.subtract)
                    nc.vector.tensor_tensor(xt[:], xt[:], rs_b[:], OP.mult)
                    nc.vector.tensor_scalar(
                        out=dst_bf[:, dt, :], in0=xt[:],
                        scalar1=s_sb[:, dt:dt + 1], scalar2=b_sb[:, dt:dt + 1],
                        op0=OP.mult, op1=OP.add)

            # ---- phase 0: LN1 -> hT ----
            xT_t = xT_d[:].rearrange("(dt p) t -> dt p t", p=P)
            hT = acts.tile([P, c.DT, c.QF], bf, tag="hT")
            emit_ln(lambda dt: xT_t[dt], hT, ln1s, ln1b)

            # ---- kv exchange buffers: 4 head-groups, pipelined AGs ----
            HPG = c.H // 4                  # heads per AG group
            VW = HPG * c.HD                 # V column width per group
            GK = HPG * c.HD * c.QF          # K elems per group
            GV = c.QF * VW                  # V elems per group
            GLEN = GK + GV
            SGPA = 4 // N_AG                 # subgroups per AG
            kv_ins = [dram.tile([1, SGPA * GLEN], bf, tag=f"kvi{a}",
                                name=f"kvi{a}") for a in range(N_AG)]
            kv_outs = [dram.tile([4, SGPA * GLEN], bf, tag=f"kvo{a}",
                                 name=f"kvo{a}") for a in range(N_AG)]
            kvi = lambda g: kv_ins[g // SGPA][0][(g % SGPA) * GLEN:
                                                 (g % SGPA + 1) * GLEN]
            kvo = lambda g: kv_outs[g // SGPA][:, (g % SGPA) * GLEN:
                                              (g % SGPA + 1) * GLEN]

            x1_dr = dram.tile([c.DT, P, c.QF], f32, tag="x1dr")
            f_dr = dram.tile([c.DT, P, c.QF], f32, tag="fdr")

            # ---- phase 1+2+3: per head-group K, V, then AllGather ----
            for g in range(4):
                for hh in range(HPG):
                    h = HPG * g + hh
                    wk_sb = streams.tile([P, c.DT, c.HD], bf, tag="wstream")
                    nc.sync.dma_start(wk_sb[:],
                                      wk_d[h].rearrange("dt p hd -> p dt hd"))
                    kps = ps512.tile([P, c.QF], f32, tag="mm")
                    for dt in range(c.DT):
                        nc.tensor.matmul(kps, wk_sb[:, dt, :], hT[:, dt, :],
                                         start=(dt == 0), stop=(dt == c.DT - 1))
                    k_tmp = drains.tile([P, c.QF], bf, tag="ktmp")
                    nc.any.tensor_copy(out=k_tmp[:], in_=kps[:])
                    ksec = kvi(g)[hh * c.HD * c.QF:(hh + 1) * c.HD * c.QF]
                    nc.sync.dma_start(ksec.rearrange("(p t) -> p t", p=P), k_tmp[:])
                # V columns for this head group (512 wide)
                wv_sb = wvs.tile([P, c.DT, VW], bf, tag="wvhc")
                nc.scalar.dma_start(
                    wv_sb[:],
                    wv_d[:, :, g * VW:(g + 1) * VW].rearrange("dt p e -> p dt e"))
                vsec = kvi(g)[GK:GK + GV]
                vsec_t = vsec.rearrange("(tt p e) -> p tt e", p=P, tt=c.TPC)
                for tt in range(c.TPC):
                    vps = ps512.tile([P, VW], f32, tag="mm")
                    for dt in range(c.DT):
                        nc.tensor.matmul(vps, hT[:, dt, tt * P:(tt + 1) * P],
                                         wv_sb[:, dt, :],
                                         start=(dt == 0), stop=(dt == c.DT - 1))
                    v_dr = drains.tile([P, VW], bf, tag="vdr")
                    nc.any.tensor_copy(out=v_dr[:], in_=vps[:])
                    nc.sync.dma_start(vsec_t[:, tt, :], v_dr[:])
                if (g + 1) % SGPA == 0:
                    a = g // SGPA
                    nc.gpsimd.collective_compute(
                        "AllGather", OP.bypass,
                        replica_groups=[[0, 1, 2, 3], [4, 5, 6, 7]],
                        ins=[kv_ins[a].opt()],
                        outs=[kv_outs[a].opt()],
                    )

            # ---- phase 3b: Q projection (overlaps the AllGathers) ----
            qT_all = acts.tile([P, c.H, c.QF], bf, tag="qo")
            for h in range(c.H):
                wq_sb = streams.tile([P, c.DT, c.HD], bf, tag="wstream")
                nc.sync.dma_start(wq_sb[:], wq_d[h].rearrange("dt p hd -> p dt hd"))
                qps = ps512.tile([P, c.QF], f32, tag="mm")
                for dt in range(c.DT):
                    nc.tensor.matmul(qps, wq_sb[:, dt, :], hT[:, dt, :],
                                     start=(dt == 0), stop=(dt == c.DT - 1))
                nc.vector.tensor_copy(out=qT_all[:, h, :], in_=qps[:])

            # ---- phase 4: attention ----
            attn_all = acts.tile([P, c.H, c.QF], bf, tag="ah")
            isq = float(c.HD) ** -0.5
            for h in range(c.H):
                g, hh = h // HPG, h % HPG
                k_sb = kvs.tile([P, 4, c.QF], bf, tag="ksb")
                ksec_o = kvo(g)[:, hh * c.HD * c.QF:(hh + 1) * c.HD * c.QF]
                nc.sync.dma_start(k_sb[:],
                                  ksec_o.rearrange("r (p t) -> p r t", p=P))
                v_sb = kvs.tile([P, 4, c.TPC, c.HD], bf, tag="vsb")
                for r in range(4):
                    vsec_o = kvo(g)[r, GK:GK + GV]
                    nc.scalar.dma_start(
                        v_sb[:, r],
                        vsec_o.rearrange("(tt p e) -> p tt e", p=P, tt=c.TPC)
                        [:, :, hh * c.HD:(hh + 1) * c.HD])
                for ci in range(2):
                    qs = qT_all[:, h, ci * c.CH:(ci + 1) * c.CH]
                    aps = psat.tile([P, c.CH], f32, tag="at")
                    sps = psvec.tile([1, c.CH], f32, tag="vec")
                    nkt = c.SP // 2 if ci == 0 else c.SP
                    for kt in range(nkt):
                        ch = kt // c.CHP
                        r = chunk_rank(ch)
                        sl = chunk_slot(ch)
                        kin = kt % c.CHP
                        toff = sl * c.CH + kin * P
                        scps = pssc.tile([P, c.CH], f32, tag="sc")
                        nc.tensor.matmul(scps, k_sb[:, r, toff:toff + P], qs,
                                         start=True, stop=True)
                        e_bf = etile.tile([P, c.CH], bf, tag="ebf")
                        nc.scalar.activation(e_bf[:], scps[:], AF.Exp, scale=isq)
                        nc.vector.tensor_mul(e_bf[:], e_bf[:], mask_sb[:, ci, kt, :])
                        vidx = sl * c.CHP + kin
                        nc.tensor.matmul(aps, v_sb[:, r, vidx, :], e_bf[:],
                                         start=(kt == 0), stop=(kt == nkt - 1))
                        nc.tensor.matmul(sps, ones_b, e_bf[:],
                                         start=(kt == 0), stop=(kt == nkt - 1))
                    rec = recs.tile([1, c.CH], f32, tag="rec")
                    nc.vector.reciprocal(rec[:], sps[:])
                    rec_bp = pssc.tile([P, c.CH], f32, tag="sc")
                    nc.tensor.matmul(rec_bp, ones_r, rec[:], start=True, stop=True)
                    rec_b = bcasts.tile([P, c.CH], f32, tag="rec_b")
                    nc.any.tensor_copy(out=rec_b[:], in_=rec_bp[:])
                    nc.vector.tensor_tensor(
                        attn_all[:, h, ci * c.CH:(ci + 1) * c.CH],
                        aps[:], rec_b[:], OP.mult)

            # ---- phase 5: o1T = sum_h wo1_h^T @ attnT_h ----
            o1T = acts.tile([P, c.DT, c.QF], bf, tag="qo")
            for dt in range(c.DT):
                w_sb = streams.tile([P, c.H, P], bf, tag="wstream")
                (nc.scalar if dt % 2 == 0 else nc.sync).dma_start(
                    w_sb[:], wo1_d[dt].rearrange("h hd e -> hd h e"))
                ops = ps512.tile([P, c.QF], f32, tag="mm")
                for h in range(c.H):
                    nc.tensor.matmul(ops, w_sb[:, h, :], attn_all[:, h, :],
                                     start=(h == 0), stop=(h == c.H - 1))
                nc.any.tensor_copy(out=o1T[:, dt, :], in_=ops[:])

            # ---- phase 6: o2T + residual -> x1 (DRAM) ----
            for et in range(c.DT):
                w_sb = streams.tile([P, c.DT, P], bf, tag="wstream")
                (nc.scalar if et % 2 == 0 else nc.sync).dma_start(
                    w_sb[:], wo2_d[et].rearrange("dt d e -> d dt e"))
                ops = ps512.tile([P, c.QF], f32, tag="mm")
                for dt in range(c.DT):
                    nc.tensor.matmul(ops, w_sb[:, dt, :], o1T[:, dt, :],
                                     start=(dt == 0), stop=(dt == c.DT - 1))
                x1_t = drains.tile([P, c.QF], f32, tag="x1t")
                nc.vector.tensor_tensor(x1_t[:], ops[:], hT[:, et, :], OP.add)
                nc.sync.dma_start(x1_dr[et], x1_t[:])

            # ---- phase 7: LN2 -> h2T ----
            h2T = acts.tile([P, c.DT, c.QF], bf, tag="ah")
            emit_ln(lambda dt: x1_dr[dt], h2T, ln2s, ln2b)

            # ---- phases 8+9: FFN by d_ff halves (N=512, partial-f via DRAM) ----
            outT_t = outT_d[:].rearrange("(et p) t -> et p t", p=P)
            FH = c.FT // 2
            FQ = min(16, FH)
            for fh in range(2):
                zT = acts.tile([P, FH, c.QF], bf, tag="big")
                for fi in range(FH):
                    ft = fh * FH + fi
                    w_sb = streams.tile([P, c.DT, P], bf, tag="wstream")
                    nc.scalar.dma_start(w_sb[:],
                                        fw1_d[ft].rearrange("dt d f -> d dt f"))
                    zps = ps512.tile([P, c.QF], f32, tag="mm")
                    for dt in range(c.DT):
                        nc.tensor.matmul(zps, w_sb[:, dt, :], h2T[:, dt, :],
                                         start=(dt == 0), stop=(dt == c.DT - 1))
                    nc.scalar.activation(zT[:, fi, :], zps[:], AF.Relu,
                                         bias=fb1[:, ft:ft + 1])
                for et in range(c.DT):
                    fps = ps512.tile([P, c.QF], f32, tag="mm")
                    for qt in range(FH // FQ):
                        w_sb = fw2s.tile([P, FQ, P], bf, tag="fw2s")
                        nc.sync.dma_start(
                            w_sb[:],
                            fw2_d[et, fh * FH + qt * FQ:fh * FH + (qt + 1) * FQ]
                            .rearrange("ft f e -> f ft e"))
                        for qi in range(FQ):
                            fi = qt * FQ + qi
                            nc.tensor.matmul(fps, w_sb[:, qi, :], zT[:, fi, :],
                                             start=(fi == 0),
                                             stop=(fi == FH - 1))
                    if fh == 0:
                        fp_t = drains.tile([P, c.QF], f32, tag="odrain")
                        nc.any.tensor_copy(out=fp_t[:], in_=fps[:])
                        nc.sync.dma_start(f_dr[et], fp_t[:])
                    else:
                        fpart = xs.tile([P, c.QF], f32, tag="xs")
                        nc.sync.dma_start(fpart[:], f_dr[et])
                        tmp_o = drains.tile([P, c.QF], f32, tag="odrain")
                        nc.vector.tensor_scalar(out=tmp_o[:], in0=fps[:],
                                                scalar1=fb2[:, et:et + 1],
                                                scalar2=None, op0=OP.add)
                        nc.vector.tensor_tensor(tmp_o[:], tmp_o[:], fpart[:],
                                                OP.add)
                        nc.vector.tensor_tensor(tmp_o[:], tmp_o[:],
                                                h2T[:, et, :], OP.add)
                        nc.sync.dma_start(outT_t[et], tmp_o[:])

    nc.compile()
    return nc


def prepare_inputs(cfg: Cfg, inputs):
    """Full inputs -> per-core in_maps."""
    c = cfg
    x = np.asarray(inputs["x"], np.float32)
    wq = np.asarray(inputs["wq"], np.float32).reshape(c.D, c.D)
    wk = np.asarray(inputs["wk"], np.float32).reshape(c.D, c.D)
    wv = np.asarray(inputs["wv"], np.float32).reshape(c.D, c.D)
    wo1 = np.asarray(inputs["wo1"], np.float32)   # [H, HD, D]
    wo2 = np.asarray(inputs["wo2"], np.float32)
    fw1 = np.asarray(inputs["fw1"], np.float32)
    fw2 = np.asarray(inputs["fw2"], np.float32)

    def pt(a):  # [D, X] -> [DT, P, X] tiles, bf16
        return np.ascontiguousarray(a.reshape(-1, P, a.shape[1])).astype(BF16)

    shared = {
        "wq": np.ascontiguousarray(
            wq.reshape(c.DT, P, c.H, c.HD).transpose(2, 0, 1, 3)).astype(BF16),
        "wk": np.ascontiguousarray(
            wk.reshape(c.DT, P, c.H, c.HD).transpose(2, 0, 1, 3)).astype(BF16),
        "wv": pt(wv),
        "wo1": np.ascontiguousarray(
            wo1.reshape(c.H, c.HD, c.DT, P).transpose(2, 0, 1, 3)).astype(BF16),
        "wo2": np.ascontiguousarray(
            wo2.reshape(c.DT, P, c.DT, P).transpose(2, 0, 1, 3)).astype(BF16),
        "fw1": np.ascontiguousarray(
            fw1.reshape(c.DT, P, c.FT, P).transpose(2, 0, 1, 3)).astype(BF16),
        "fw2": np.ascontiguousarray(
            fw2.reshape(c.FT, P, c.DT, P).transpose(2, 0, 1, 3)).astype(BF16),
        "ln1s": np.ascontiguousarray(
            np.asarray(inputs["ln1_scale"], np.float32).reshape(c.DT, P).T),
        "ln1b": np.ascontiguousarray(
            np.asarray(inputs["ln1_bias"], np.float32).reshape(c.DT, P).T),
        "ln2s": np.ascontiguousarray(
            np.asarray(inputs["ln2_scale"], np.float32).reshape(c.DT, P).T),
        "ln2b": np.ascontiguousarray(
            np.asarray(inputs["ln2_bias"], np.float32).reshape(c.DT, P).T),
        "fb1": np.ascontiguousarray(
            np.asarray(inputs["fb1"], np.float32).reshape(c.FT, P).T),
        "fb2": np.ascontiguousarray(
            np.asarray(inputs["fb2"], np.float32).reshape(c.DT, P).T),
    }

    in_maps = []
    for core in range(8):
        b, j = core // 4, core % 4
        ca, cb = j, 7 - j
        rows_a = slice(c.CH * ca, c.CH * (ca + 1))
        rows_b = slice(c.CH * cb, c.CH * (cb + 1))
        xc = np.concatenate([x[b, rows_a], x[b, rows_b]], axis=0)  # [QF, D]
        xT = np.ascontiguousarray(xc.T)
        mask = np.zeros((2, c.SP, P, c.CH), np.float32)
        for ci, ch in enumerate((ca, cb)):
            q0 = c.CH * ch
            qpos = q0 + np.arange(c.CH)[None, :]
            for kt in range(c.SP):
                kvpos = kt * P + np.arange(P)[:, None]
                mask[ci, kt] = (kvpos <= qpos)
        m = dict(shared)
        m["xT"] = xT
        m["mask"] = mask.astype(BF16)
        in_maps.append(m)
    return in_maps


def assemble_output(cfg: Cfg, results):
    c = cfg
    out = np.empty((c.B, c.S, c.D), np.float32)
    for core in range(8):
        b, j = core // 4, core % 4
        oT = results[core]["outT"]  # [D, QF]
        out[b, c.CH * j:c.CH * (j + 1)] = oT[:, :c.CH].T
        out[b, c.CH * (7 - j):c.CH * (8 - j)] = oT[:, c.CH:].T
    return out


_CACHE = {}


def kernel(**inputs):
    cfg = Cfg()
    if "nc" not in _CACHE:
        _CACHE["nc"] = build_program(cfg)
    nc = _CACHE["nc"]
    in_maps = prepare_inputs(cfg, inputs)
    res = run_bass_kernel_spmd(nc, in_maps, core_ids=list(range(8)))
    return assemble_output(cfg, res.results)
